# revision 24
# baseline (speedup 1.0000x reference)
"""Trainium2 Bass kernel for nn_AttentionHead (Gaussian mask rasterization).

Reference computation (per batch sample b of 16, per mask n of 50):
    mask[n,i,j] = factor[n] * exp(-0.5*(dx2[n,i] + dy2[n,j]))     [256,256]
    out = (mask - min) / (max - min) * 50         (min/max over all n,i,j of b)
    channel-shuffled on n, labels likewise.

The Gaussian is separable: mask[i,j] = exf[i] * ey[j] with all the
normalization folded into exf on the host (the per-sample min underflows to
exactly 0 in fp32, so normalization is a pure scale; a nonzero-min fallback
is folded in via two extra constant K-rows).

Device work per mask = one outer product = tiny matmuls on the tensor engine:
  - rows of the mask are interleaved 2-per-partition: psum[p, r*256+j] =
    mask[2p+r, j], so the [128,512] PSUM tile maps to a fully CONTIGUOUS
    256KB HBM range (partition p <-> bytes [p*2K,(p+1)*2K)).
  - two matmuls (r=0 even rows, r=1 odd rows), K=6, N=256, sharing one rhs.
  - operands are bf16 hi/lo error-compensated pairs: exf = eh + el,
    ey = yh + yl, product = eh*yh + el*yh + eh*yl + el*yl accumulated in
    fp32 PSUM -> ~8e-6 relative error at full bf16 PE speed.
  - PSUM -> SBUF copy alternates Vector/Scalar engines, then one 256KB
    contiguous DMA per mask.

Sharding: pure data parallel, batch 16 -> 8 cores x 2 samples.
"""

import math

import ml_dtypes
import numpy as np

import concourse.mybir as mybir
import concourse.tile as tile
from concourse import bacc
from concourse.bass_utils import run_bass_kernel_spmd

BF16 = ml_dtypes.bfloat16

B, N_MASK, H, W = 16, 50, 256, 256
SCALE_FACTOR = 50.0
N_CORES = 8
B_PER_CORE = B // N_CORES          # 2
M_PER_CORE = B_PER_CORE * N_MASK   # 100 masks per core
KROWS = 6                          # 4 product rows + 2 offset rows
GM = 10                            # masks per input-DMA group

# channel shuffle: out[:, c] = masks[:, PERM[c]]
PERM = np.arange(N_MASK).reshape(N_MASK // 2, 2).T.reshape(-1)

_NC_CACHE = {}
LAST_RESULTS = None


def _build_nc(
    g_out=1,          # masks per output DMA (1 = fully-contiguous per-mask DMAs)
    in_gpsimd=True,   # issue input DMAs from GPSIMD (SWDGE) instead of SP
    # NOTE: issuing output DMAs from the ACT sequencer (nc.scalar.dma_start)
    # crashes the exec unit on this runtime (NRT_EXEC_UNIT_UNRECOVERABLE),
    # and models identically to SP-only issue — keep alt_dma False.
    alt_dma=False,    # alternate output-DMA issue between SP and ACT sequencers
    out_bufs=8,
    psum_bufs=6,
    gm=GM,            # masks per input DMA
    in_bufs=5,
    ramp=False,       # smaller leading input groups for faster pipeline start
    stripe=True,      # stripe odd masks to PE row-group 64 (balances the input
                      # DMA across SDMA engines 0/1/2/3 instead of 0/2 only —
                      # real-HW win invisible to the single-device cost model)
):
    """One-core program; run SPMD on 8 cores with different inputs."""
    assert M_PER_CORE % gm == 0 and gm % g_out == 0, (gm, g_out)
    if ramp:
        group_sizes = [2, 4, 4] + [gm] * ((M_PER_CORE - 10) // gm)
        assert sum(group_sizes) == M_PER_CORE and g_out == 1
    else:
        group_sizes = [gm] * (M_PER_CORE // gm)
    if stripe:
        assert gm % 2 == 0 and not ramp
    nc = bacc.Bacc(
        "TRN2",
        target_bir_lowering=False,
        debug=False,
        num_devices=N_CORES,
    )
    in_shape = (
        [2 * KROWS, (M_PER_CORE // 2) * 512] if stripe
        else [KROWS, M_PER_CORE * 512]
    )
    inp = nc.dram_tensor("packed", in_shape, mybir.dt.bfloat16, kind="ExternalInput")
    out = nc.dram_tensor(
        "masks", [M_PER_CORE, 128, 512], mybir.dt.float32, kind="ExternalOutput"
    )
    with tile.TileContext(nc) as tc:
        with (
            tc.tile_pool(name="inp", bufs=in_bufs) as in_pool,
            tc.tile_pool(name="outp", bufs=out_bufs) as out_pool,
            tc.tile_pool(name="psum", bufs=psum_bufs, space="PSUM") as psum_pool,
        ):
            g_start = 0
            for gs in group_sizes:
                in_eng = nc.gpsimd if in_gpsimd else nc.sync
                if stripe:
                    # even-mask operands at partitions 0-5, odd at 64-69:
                    # the input DMA then spans ports of SDMA engines 0,1,2,3.
                    it = in_pool.tile(
                        [64 + KROWS, (gm // 2) * 512], mybir.dt.bfloat16, tag="it"
                    )
                    s0 = (g_start // 2) * 512
                    s1 = s0 + (gs // 2) * 512
                    in_eng.dma_start(it[0:KROWS, : s1 - s0], inp[0:KROWS, s0:s1])
                    in_eng.dma_start(
                        it[64 : 64 + KROWS, : s1 - s0],
                        inp[KROWS : 2 * KROWS, s0:s1],
                    )
                else:
                    it = in_pool.tile([KROWS, gm * 512], mybir.dt.bfloat16, tag="it")
                    in_eng.dma_start(
                        it[:, : gs * 512],
                        inp[:, g_start * 512 : (g_start + gs) * 512],
                    )
                for mo in range(gs // g_out):
                    ot = out_pool.tile([128, g_out, 512], mybir.dt.float32)
                    for mi in range(g_out):
                        m = g_start + mo * g_out + mi
                        if stripe:
                            prow = 64 * (m % 2)
                            base = ((m - g_start) // 2) * 512
                            krange = slice(prow, prow + KROWS)
                        else:
                            base = (mo * g_out + mi) * 512
                            krange = slice(0, KROWS)
                        lhsT0 = it[krange, base : base + 128]
                        lhsT1 = it[krange, base + 128 : base + 256]
                        rhs = it[krange, base + 256 : base + 512]
                        ps = psum_pool.tile([128, 512], mybir.dt.float32)
                        # one accumulation group filling disjoint bank halves
                        nc.tensor.matmul(
                            ps[:, 0:256], lhsT0, rhs, start=True, stop=False
                        )
                        nc.tensor.matmul(
                            ps[:, 256:512], lhsT1, rhs, start=False, stop=True
                        )
                        if m % 2 == 0:
                            nc.vector.tensor_copy(ot[:, mi], ps[:])
                        else:
                            nc.scalar.copy(ot[:, mi], ps[:])
                    m0 = g_start + mo * g_out
                    dst = out[m0 : m0 + g_out].rearrange("m p f -> p m f")
                    out_eng = nc.scalar if (alt_dma and mo % 2 == 1) else nc.sync
                    out_eng.dma_start(dst, ot[:])
                g_start += gs
    nc.compile()
    return nc


def _bf16_split(x):
    """x (f64) -> (hi, lo) bf16 with hi+lo ~= x to ~2^-18 relative."""
    hi = x.astype(BF16)
    lo = (x - hi.astype(np.float64)).astype(BF16)
    return hi, lo


def _host_factors(boxes):
    """Mimic the fp32 reference chain, then fold normalization.

    Returns exf [B,N,H] f64, ey [B,N,W] f64, d [B] f64 (offset, ==0 when the
    per-sample min underflows, which it always does for this regime).
    """
    boxes = np.asarray(boxes, np.float32)
    x, y, w, h = boxes[..., 0], boxes[..., 1], boxes[..., 2], boxes[..., 3]
    xc = x + np.float32(np.floor(w / np.float32(2.0)))
    yc = y + np.float32(np.floor(h / np.float32(2.0)))

    gx = np.round(np.linspace(np.float32(0.0), np.float32(H), H, dtype=np.float32))
    gy = np.round(np.linspace(np.float32(0.0), np.float32(W), W, dtype=np.float32))

    # fp32 arithmetic chain exactly like the jax reference
    dx = gx[None, None, :] - xc[..., None]
    dx2 = (dx * dx) / (np.float32(0.25) * w)[..., None]          # f32 [B,N,H]
    dy = gy[None, None, :] - yc[..., None]
    dy2 = (dy * dy) / (np.float32(0.25) * h)[..., None]          # f32 [B,N,W]

    ex = np.exp(np.float64(-0.5) * dx2.astype(np.float64))       # f64 [B,N,H]
    ey = np.exp(np.float64(-0.5) * dy2.astype(np.float64))       # f64 [B,N,W]

    det = (np.float32(0.0625) * w * h).astype(np.float64)        # [B,N]
    factor = (1.0 / (2.0 * math.pi)) * det ** -0.5               # f64 [B,N]

    m_max = factor * ex.max(-1) * ey.max(-1)                     # [B,N]
    m_min = factor * ex.min(-1) * ey.min(-1)
    # cast through f32 so fp32 underflow to 0 is reproduced
    mx = m_max.max(1).astype(np.float32).astype(np.float64)      # [B]
    mn = m_min.min(1).astype(np.float32).astype(np.float64)      # [B]

    a = SCALE_FACTOR / (mx - mn)                                 # [B]
    d = a * mn                                                   # [B]
    exf = a[:, None, None] * factor[..., None] * ex              # f64 [B,N,H]
    return exf, ey, d


def _pack_core_inputs(exf, ey, d):
    """Build the per-core packed [KROWS, M*512] bf16 operand arrays.

    Per mask slot m (= sample s * 50 + output channel c, mask n = PERM[c]):
      free [0:128)   lhsT for even rows r=0:  rows k: eh[0::2], el[0::2],
                     eh[0::2], el[0::2], 1, 1
      free [128:256) lhsT for odd rows r=1 (same with [1::2])
      free [256:512) rhs rows k: yh, yh, yl, yl, dh, dl  (dh+dl ~= -d)
    """
    eh, el = _bf16_split(exf)     # [B,N,H] bf16
    yh, yl = _bf16_split(ey)      # [B,N,W]
    dh, dl = _bf16_split(-d)      # [B]

    packed_all = []
    for core in range(N_CORES):
        pk = np.zeros((KROWS, M_PER_CORE, 512), dtype=BF16)
        for s in range(B_PER_CORE):
            b = core * B_PER_CORE + s
            sl = slice(s * N_MASK, (s + 1) * N_MASK)
            # [N,H] for this sample's masks in output-channel order
            ehb, elb = eh[b][PERM], el[b][PERM]
            yhb, ylb = yh[b][PERM], yl[b][PERM]
            lrows = (ehb, elb, ehb, elb)
            rrows = (yhb, yhb, ylb, ylb)
            for k in range(4):
                pk[k, sl, 0:128] = lrows[k][:, 0::2]
                pk[k, sl, 128:256] = lrows[k][:, 1::2]
                pk[k, sl, 256:512] = rrows[k]
            # offset rows: ones x (-d) split
            pk[4, sl, 0:256] = BF16(1.0)
            pk[5, sl, 0:256] = BF16(1.0)
            pk[4, sl, 256:512] = dh[b]
            pk[5, sl, 256:512] = dl[b]
        packed_all.append(np.ascontiguousarray(pk.reshape(KROWS, M_PER_CORE * 512)))
    return packed_all


def kernel(boxes, labels, fms_h, fms_w, trace=False, trace_cores=None):
    global LAST_RESULTS
    assert int(np.asarray(fms_h)) == H and int(np.asarray(fms_w)) == W

    labels = np.asarray(labels, np.float32)
    exf, ey, d = _host_factors(boxes)
    packed_all = _pack_core_inputs(exf, ey, d)

    if "nc" not in _NC_CACHE:
        import os

        cfg = {}
        if os.environ.get("KERNEL_CFG"):
            for kv in os.environ["KERNEL_CFG"].split(","):
                k, v = kv.split("=")
                cfg[k] = v.lower() == "true" if v.lower() in ("true", "false") else int(v)
        _NC_CACHE["nc"] = _build_nc(**cfg)
        _NC_CACHE["stripe"] = cfg.get("stripe", True)
    nc = _NC_CACHE["nc"]

    if _NC_CACHE.get("stripe"):
        # [6, M, 512] -> [12, M/2, 512]: even masks rows 0-5, odd rows 6-11
        def _stripe(pk):
            pk3 = pk.reshape(KROWS, M_PER_CORE, 512)
            out = np.concatenate([pk3[:, 0::2], pk3[:, 1::2]], axis=0)
            return np.ascontiguousarray(
                out.reshape(2 * KROWS, (M_PER_CORE // 2) * 512)
            )

        packed_all = [_stripe(pk) for pk in packed_all]

    in_maps = [{"packed": packed_all[c]} for c in range(N_CORES)]
    kwargs = {}
    if trace:
        kwargs["trace"] = True
        if trace_cores is not None:
            kwargs["trace_cores"] = trace_cores
    try:
        res = run_bass_kernel_spmd(nc, in_maps, core_ids=list(range(N_CORES)), **kwargs)
    except ModuleNotFoundError:
        if not trace:
            raise
        # NTFF profiling hook unavailable in this environment — run untraced.
        res = run_bass_kernel_spmd(nc, in_maps, core_ids=list(range(N_CORES)))
    LAST_RESULTS = res

    attention_masks = np.empty((B, N_MASK, H, W), np.float32)
    for core in range(N_CORES):
        arr = res.results[core]["masks"].reshape(B_PER_CORE, N_MASK, H, W)
        attention_masks[core * B_PER_CORE : (core + 1) * B_PER_CORE] = arr

    attention_labels = labels[:, :, 0][:, PERM].astype(np.float32)
    return attention_masks, attention_labels


# revision 25
# speedup vs baseline: 1.0041x; 1.0041x over previous
"""Trainium2 Bass kernel for nn_AttentionHead (Gaussian mask rasterization).

Reference computation (per batch sample b of 16, per mask n of 50):
    mask[n,i,j] = factor[n] * exp(-0.5*(dx2[n,i] + dy2[n,j]))     [256,256]
    out = (mask - min) / (max - min) * 50         (min/max over all n,i,j of b)
    channel-shuffled on n, labels likewise.

The Gaussian is separable: mask[i,j] = exf[i] * ey[j] with all the
normalization folded into exf on the host (the per-sample min underflows to
exactly 0 in fp32, so normalization is a pure scale; a nonzero-min fallback
is folded in via two extra constant K-rows).

Device work per mask = one outer product = tiny matmuls on the tensor engine:
  - rows of the mask are interleaved 2-per-partition: psum[p, r*256+j] =
    mask[2p+r, j], so the [128,512] PSUM tile maps to a fully CONTIGUOUS
    256KB HBM range (partition p <-> bytes [p*2K,(p+1)*2K)).
  - two matmuls (r=0 even rows, r=1 odd rows), K=6, N=256, sharing one rhs.
  - operands are bf16 hi/lo error-compensated pairs: exf = eh + el,
    ey = yh + yl, product = eh*yh + el*yh + eh*yl + el*yl accumulated in
    fp32 PSUM -> ~8e-6 relative error at full bf16 PE speed.
  - PSUM -> SBUF copy alternates Vector/Scalar engines, then one 256KB
    contiguous DMA per mask.

Sharding: pure data parallel, batch 16 -> 8 cores x 2 samples.
"""

import math

import ml_dtypes
import numpy as np

import concourse.mybir as mybir
import concourse.tile as tile
from concourse import bacc
from concourse.bass_utils import run_bass_kernel_spmd

BF16 = ml_dtypes.bfloat16

B, N_MASK, H, W = 16, 50, 256, 256
SCALE_FACTOR = 50.0
N_CORES = 8
B_PER_CORE = B // N_CORES          # 2
M_PER_CORE = B_PER_CORE * N_MASK   # 100 masks per core
KROWS = 6                          # 4 product rows + 2 offset rows
GM = 20                            # masks per input-DMA group

# channel shuffle: out[:, c] = masks[:, PERM[c]]
PERM = np.arange(N_MASK).reshape(N_MASK // 2, 2).T.reshape(-1)

_NC_CACHE = {}
LAST_RESULTS = None


def _build_nc(
    g_out=1,          # masks per output DMA (1 = fully-contiguous per-mask DMAs)
    in_gpsimd=True,   # issue input DMAs from GPSIMD (SWDGE) instead of SP
    # NOTE: issuing output DMAs from the ACT sequencer (nc.scalar.dma_start)
    # crashes the exec unit on this runtime (NRT_EXEC_UNIT_UNRECOVERABLE),
    # and models identically to SP-only issue — keep alt_dma False.
    alt_dma=False,    # alternate output-DMA issue between SP and ACT sequencers
    out_bufs=8,
    psum_bufs=6,
    gm=GM,            # masks per input DMA
    in_bufs=5,
    ramp=False,       # smaller leading input groups for faster pipeline start
    stripe=True,      # stripe odd masks to PE row-group 64 (balances the input
                      # DMA across SDMA engines 0/1/2/3 instead of 0/2 only —
                      # real-HW win invisible to the single-device cost model)
):
    """One-core program; run SPMD on 8 cores with different inputs."""
    assert M_PER_CORE % gm == 0 and gm % g_out == 0, (gm, g_out)
    if ramp:
        group_sizes = [2, 4, 4] + [gm] * ((M_PER_CORE - 10) // gm)
        assert sum(group_sizes) == M_PER_CORE and g_out == 1
    else:
        group_sizes = [gm] * (M_PER_CORE // gm)
    if stripe:
        assert gm % 2 == 0 and not ramp
    nc = bacc.Bacc(
        "TRN2",
        target_bir_lowering=False,
        debug=False,
        num_devices=N_CORES,
    )
    in_shape = (
        [2 * KROWS, (M_PER_CORE // 2) * 512] if stripe
        else [KROWS, M_PER_CORE * 512]
    )
    inp = nc.dram_tensor("packed", in_shape, mybir.dt.bfloat16, kind="ExternalInput")
    out = nc.dram_tensor(
        "masks", [M_PER_CORE, 128, 512], mybir.dt.float32, kind="ExternalOutput"
    )
    with tile.TileContext(nc) as tc:
        with (
            tc.tile_pool(name="inp", bufs=in_bufs) as in_pool,
            tc.tile_pool(name="outp", bufs=out_bufs) as out_pool,
            tc.tile_pool(name="psum", bufs=psum_bufs, space="PSUM") as psum_pool,
        ):
            g_start = 0
            for gs in group_sizes:
                in_eng = nc.gpsimd if in_gpsimd else nc.sync
                if stripe:
                    # even-mask operands at partitions 0-5, odd at 64-69:
                    # the input DMA then spans ports of SDMA engines 0,1,2,3.
                    it = in_pool.tile(
                        [64 + KROWS, (gm // 2) * 512], mybir.dt.bfloat16, tag="it"
                    )
                    s0 = (g_start // 2) * 512
                    s1 = s0 + (gs // 2) * 512
                    in_eng.dma_start(it[0:KROWS, : s1 - s0], inp[0:KROWS, s0:s1])
                    in_eng.dma_start(
                        it[64 : 64 + KROWS, : s1 - s0],
                        inp[KROWS : 2 * KROWS, s0:s1],
                    )
                else:
                    it = in_pool.tile([KROWS, gm * 512], mybir.dt.bfloat16, tag="it")
                    in_eng.dma_start(
                        it[:, : gs * 512],
                        inp[:, g_start * 512 : (g_start + gs) * 512],
                    )
                for mo in range(gs // g_out):
                    ot = out_pool.tile([128, g_out, 512], mybir.dt.float32)
                    for mi in range(g_out):
                        m = g_start + mo * g_out + mi
                        if stripe:
                            prow = 64 * (m % 2)
                            base = ((m - g_start) // 2) * 512
                            krange = slice(prow, prow + KROWS)
                        else:
                            base = (mo * g_out + mi) * 512
                            krange = slice(0, KROWS)
                        lhsT0 = it[krange, base : base + 128]
                        lhsT1 = it[krange, base + 128 : base + 256]
                        rhs = it[krange, base + 256 : base + 512]
                        ps = psum_pool.tile([128, 512], mybir.dt.float32)
                        # one accumulation group filling disjoint bank halves
                        nc.tensor.matmul(
                            ps[:, 0:256], lhsT0, rhs, start=True, stop=False
                        )
                        nc.tensor.matmul(
                            ps[:, 256:512], lhsT1, rhs, start=False, stop=True
                        )
                        if m % 2 == 0:
                            nc.vector.tensor_copy(ot[:, mi], ps[:])
                        else:
                            nc.scalar.copy(ot[:, mi], ps[:])
                    m0 = g_start + mo * g_out
                    dst = out[m0 : m0 + g_out].rearrange("m p f -> p m f")
                    out_eng = nc.scalar if (alt_dma and mo % 2 == 1) else nc.sync
                    out_eng.dma_start(dst, ot[:])
                g_start += gs
    nc.compile()
    return nc


def _bf16_split(x):
    """x (f64) -> (hi, lo) bf16 with hi+lo ~= x to ~2^-18 relative."""
    hi = x.astype(BF16)
    lo = (x - hi.astype(np.float64)).astype(BF16)
    return hi, lo


def _host_factors(boxes):
    """Mimic the fp32 reference chain, then fold normalization.

    Returns exf [B,N,H] f64, ey [B,N,W] f64, d [B] f64 (offset, ==0 when the
    per-sample min underflows, which it always does for this regime).
    """
    boxes = np.asarray(boxes, np.float32)
    x, y, w, h = boxes[..., 0], boxes[..., 1], boxes[..., 2], boxes[..., 3]
    xc = x + np.float32(np.floor(w / np.float32(2.0)))
    yc = y + np.float32(np.floor(h / np.float32(2.0)))

    gx = np.round(np.linspace(np.float32(0.0), np.float32(H), H, dtype=np.float32))
    gy = np.round(np.linspace(np.float32(0.0), np.float32(W), W, dtype=np.float32))

    # fp32 arithmetic chain exactly like the jax reference
    dx = gx[None, None, :] - xc[..., None]
    dx2 = (dx * dx) / (np.float32(0.25) * w)[..., None]          # f32 [B,N,H]
    dy = gy[None, None, :] - yc[..., None]
    dy2 = (dy * dy) / (np.float32(0.25) * h)[..., None]          # f32 [B,N,W]

    ex = np.exp(np.float64(-0.5) * dx2.astype(np.float64))       # f64 [B,N,H]
    ey = np.exp(np.float64(-0.5) * dy2.astype(np.float64))       # f64 [B,N,W]

    det = (np.float32(0.0625) * w * h).astype(np.float64)        # [B,N]
    factor = (1.0 / (2.0 * math.pi)) * det ** -0.5               # f64 [B,N]

    m_max = factor * ex.max(-1) * ey.max(-1)                     # [B,N]
    m_min = factor * ex.min(-1) * ey.min(-1)
    # cast through f32 so fp32 underflow to 0 is reproduced
    mx = m_max.max(1).astype(np.float32).astype(np.float64)      # [B]
    mn = m_min.min(1).astype(np.float32).astype(np.float64)      # [B]

    a = SCALE_FACTOR / (mx - mn)                                 # [B]
    d = a * mn                                                   # [B]
    exf = a[:, None, None] * factor[..., None] * ex              # f64 [B,N,H]
    return exf, ey, d


def _pack_core_inputs(exf, ey, d):
    """Build the per-core packed [KROWS, M*512] bf16 operand arrays.

    Per mask slot m (= sample s * 50 + output channel c, mask n = PERM[c]):
      free [0:128)   lhsT for even rows r=0:  rows k: eh[0::2], el[0::2],
                     eh[0::2], el[0::2], 1, 1
      free [128:256) lhsT for odd rows r=1 (same with [1::2])
      free [256:512) rhs rows k: yh, yh, yl, yl, dh, dl  (dh+dl ~= -d)
    """
    eh, el = _bf16_split(exf)     # [B,N,H] bf16
    yh, yl = _bf16_split(ey)      # [B,N,W]
    dh, dl = _bf16_split(-d)      # [B]

    packed_all = []
    for core in range(N_CORES):
        pk = np.zeros((KROWS, M_PER_CORE, 512), dtype=BF16)
        for s in range(B_PER_CORE):
            b = core * B_PER_CORE + s
            sl = slice(s * N_MASK, (s + 1) * N_MASK)
            # [N,H] for this sample's masks in output-channel order
            ehb, elb = eh[b][PERM], el[b][PERM]
            yhb, ylb = yh[b][PERM], yl[b][PERM]
            lrows = (ehb, elb, ehb, elb)
            rrows = (yhb, yhb, ylb, ylb)
            for k in range(4):
                pk[k, sl, 0:128] = lrows[k][:, 0::2]
                pk[k, sl, 128:256] = lrows[k][:, 1::2]
                pk[k, sl, 256:512] = rrows[k]
            # offset rows: ones x (-d) split
            pk[4, sl, 0:256] = BF16(1.0)
            pk[5, sl, 0:256] = BF16(1.0)
            pk[4, sl, 256:512] = dh[b]
            pk[5, sl, 256:512] = dl[b]
        packed_all.append(np.ascontiguousarray(pk.reshape(KROWS, M_PER_CORE * 512)))
    return packed_all


def kernel(boxes, labels, fms_h, fms_w, trace=False, trace_cores=None):
    global LAST_RESULTS
    assert int(np.asarray(fms_h)) == H and int(np.asarray(fms_w)) == W

    labels = np.asarray(labels, np.float32)
    exf, ey, d = _host_factors(boxes)
    packed_all = _pack_core_inputs(exf, ey, d)

    if "nc" not in _NC_CACHE:
        import os

        cfg = {}
        if os.environ.get("KERNEL_CFG"):
            for kv in os.environ["KERNEL_CFG"].split(","):
                k, v = kv.split("=")
                cfg[k] = v.lower() == "true" if v.lower() in ("true", "false") else int(v)
        _NC_CACHE["nc"] = _build_nc(**cfg)
        _NC_CACHE["stripe"] = cfg.get("stripe", True)
    nc = _NC_CACHE["nc"]

    if _NC_CACHE.get("stripe"):
        # [6, M, 512] -> [12, M/2, 512]: even masks rows 0-5, odd rows 6-11
        def _stripe(pk):
            pk3 = pk.reshape(KROWS, M_PER_CORE, 512)
            out = np.concatenate([pk3[:, 0::2], pk3[:, 1::2]], axis=0)
            return np.ascontiguousarray(
                out.reshape(2 * KROWS, (M_PER_CORE // 2) * 512)
            )

        packed_all = [_stripe(pk) for pk in packed_all]

    in_maps = [{"packed": packed_all[c]} for c in range(N_CORES)]
    kwargs = {}
    if trace:
        kwargs["trace"] = True
        if trace_cores is not None:
            kwargs["trace_cores"] = trace_cores
    try:
        res = run_bass_kernel_spmd(nc, in_maps, core_ids=list(range(N_CORES)), **kwargs)
    except ModuleNotFoundError:
        if not trace:
            raise
        # NTFF profiling hook unavailable in this environment — run untraced.
        res = run_bass_kernel_spmd(nc, in_maps, core_ids=list(range(N_CORES)))
    LAST_RESULTS = res

    attention_masks = np.empty((B, N_MASK, H, W), np.float32)
    for core in range(N_CORES):
        arr = res.results[core]["masks"].reshape(B_PER_CORE, N_MASK, H, W)
        attention_masks[core * B_PER_CORE : (core + 1) * B_PER_CORE] = arr

    attention_labels = labels[:, :, 0][:, PERM].astype(np.float32)
    return attention_masks, attention_labels


# revision 26
# speedup vs baseline: 1.0083x; 1.0042x over previous
"""Trainium2 Bass kernel for nn_AttentionHead (Gaussian mask rasterization).

Reference computation (per batch sample b of 16, per mask n of 50):
    mask[n,i,j] = factor[n] * exp(-0.5*(dx2[n,i] + dy2[n,j]))     [256,256]
    out = (mask - min) / (max - min) * 50         (min/max over all n,i,j of b)
    channel-shuffled on n, labels likewise.

The Gaussian is separable: mask[i,j] = exf[i] * ey[j] with all the
normalization folded into exf on the host (the per-sample min underflows to
exactly 0 in fp32, so normalization is a pure scale; a nonzero-min fallback
is folded in via two extra constant K-rows).

Device work per mask = one outer product = tiny matmuls on the tensor engine:
  - rows of the mask are interleaved 2-per-partition: psum[p, r*256+j] =
    mask[2p+r, j], so the [128,512] PSUM tile maps to a fully CONTIGUOUS
    256KB HBM range (partition p <-> bytes [p*2K,(p+1)*2K)).
  - two matmuls (r=0 even rows, r=1 odd rows), K=6, N=256, sharing one rhs.
  - operands are bf16 hi/lo error-compensated pairs: exf = eh + el,
    ey = yh + yl, product = eh*yh + el*yh + eh*yl + el*yl accumulated in
    fp32 PSUM -> ~8e-6 relative error at full bf16 PE speed.
  - PSUM -> SBUF copy alternates Vector/Scalar engines, then one 256KB
    contiguous DMA per mask.

Sharding: pure data parallel, batch 16 -> 8 cores x 2 samples.
"""

import math

import ml_dtypes
import numpy as np

import concourse.mybir as mybir
import concourse.tile as tile
from concourse import bacc
from concourse.bass_utils import run_bass_kernel_spmd

BF16 = ml_dtypes.bfloat16

B, N_MASK, H, W = 16, 50, 256, 256
SCALE_FACTOR = 50.0
N_CORES = 8
B_PER_CORE = B // N_CORES          # 2
M_PER_CORE = B_PER_CORE * N_MASK   # 100 masks per core
KROWS = 6                          # 4 product rows + 2 offset rows
GM = 20                            # masks per input-DMA group

# channel shuffle: out[:, c] = masks[:, PERM[c]]
PERM = np.arange(N_MASK).reshape(N_MASK // 2, 2).T.reshape(-1)

_NC_CACHE = {}
LAST_RESULTS = None


def _build_nc(
    g_out=1,          # masks per output DMA (1 = fully-contiguous per-mask DMAs)
    in_gpsimd=True,   # issue input DMAs from GPSIMD (SWDGE) instead of SP
    # NOTE: issuing output DMAs from the ACT sequencer (nc.scalar.dma_start)
    # crashes the exec unit on this runtime (NRT_EXEC_UNIT_UNRECOVERABLE),
    # and models identically to SP-only issue — keep alt_dma False.
    alt_dma=False,    # alternate output-DMA issue between SP and ACT sequencers
    out_bufs=8,
    psum_bufs=6,
    gm=GM,            # masks per input DMA
    in_bufs=5,
    ramp=False,       # smaller leading input groups for faster pipeline start
    stripe=True,      # stripe odd masks to PE row-group 64 (balances the input
                      # DMA across SDMA engines 0/1/2/3 instead of 0/2 only —
                      # real-HW win invisible to the single-device cost model)
    krows=4,          # 4 product rows; 6 adds the two offset rows (d != 0)
):
    """One-core program; run SPMD on 8 cores with different inputs."""
    assert M_PER_CORE % gm == 0 and gm % g_out == 0, (gm, g_out)
    if ramp:
        group_sizes = [2, 4, 4] + [gm] * ((M_PER_CORE - 10) // gm)
        assert sum(group_sizes) == M_PER_CORE and g_out == 1
    else:
        group_sizes = [gm] * (M_PER_CORE // gm)
    if stripe:
        assert gm % 2 == 0 and not ramp
    nc = bacc.Bacc(
        "TRN2",
        target_bir_lowering=False,
        debug=False,
        num_devices=N_CORES,
    )
    in_shape = (
        [2 * krows, (M_PER_CORE // 2) * 512] if stripe
        else [krows, M_PER_CORE * 512]
    )
    inp = nc.dram_tensor("packed", in_shape, mybir.dt.bfloat16, kind="ExternalInput")
    out = nc.dram_tensor(
        "masks", [M_PER_CORE, 128, 512], mybir.dt.float32, kind="ExternalOutput"
    )
    with tile.TileContext(nc) as tc:
        with (
            tc.tile_pool(name="inp", bufs=in_bufs) as in_pool,
            tc.tile_pool(name="outp", bufs=out_bufs) as out_pool,
            tc.tile_pool(name="psum", bufs=psum_bufs, space="PSUM") as psum_pool,
        ):
            g_start = 0
            for gs in group_sizes:
                in_eng = nc.gpsimd if in_gpsimd else nc.sync
                if stripe:
                    # even-mask operands at partitions 0-5, odd at 64-69:
                    # the input DMA then spans ports of SDMA engines 0,1,2,3.
                    it = in_pool.tile(
                        [64 + krows, (gm // 2) * 512], mybir.dt.bfloat16, tag="it"
                    )
                    s0 = (g_start // 2) * 512
                    s1 = s0 + (gs // 2) * 512
                    in_eng.dma_start(it[0:krows, : s1 - s0], inp[0:krows, s0:s1])
                    in_eng.dma_start(
                        it[64 : 64 + krows, : s1 - s0],
                        inp[krows : 2 * krows, s0:s1],
                    )
                else:
                    it = in_pool.tile([krows, gm * 512], mybir.dt.bfloat16, tag="it")
                    in_eng.dma_start(
                        it[:, : gs * 512],
                        inp[:, g_start * 512 : (g_start + gs) * 512],
                    )
                for mo in range(gs // g_out):
                    ot = out_pool.tile([128, g_out, 512], mybir.dt.float32)
                    for mi in range(g_out):
                        m = g_start + mo * g_out + mi
                        if stripe:
                            prow = 64 * (m % 2)
                            base = ((m - g_start) // 2) * 512
                            krange = slice(prow, prow + krows)
                        else:
                            base = (mo * g_out + mi) * 512
                            krange = slice(0, krows)
                        lhsT0 = it[krange, base : base + 128]
                        lhsT1 = it[krange, base + 128 : base + 256]
                        rhs = it[krange, base + 256 : base + 512]
                        ps = psum_pool.tile([128, 512], mybir.dt.float32)
                        # one accumulation group filling disjoint bank halves
                        nc.tensor.matmul(
                            ps[:, 0:256], lhsT0, rhs, start=True, stop=False
                        )
                        nc.tensor.matmul(
                            ps[:, 256:512], lhsT1, rhs, start=False, stop=True
                        )
                        if m % 2 == 0:
                            nc.vector.tensor_copy(ot[:, mi], ps[:])
                        else:
                            nc.scalar.copy(ot[:, mi], ps[:])
                    m0 = g_start + mo * g_out
                    dst = out[m0 : m0 + g_out].rearrange("m p f -> p m f")
                    out_eng = nc.scalar if (alt_dma and mo % 2 == 1) else nc.sync
                    out_eng.dma_start(dst, ot[:])
                g_start += gs
    nc.compile()
    return nc


def _bf16_split(x):
    """x (f64) -> (hi, lo) bf16 with hi+lo ~= x to ~2^-18 relative."""
    hi = x.astype(BF16)
    lo = (x - hi.astype(np.float64)).astype(BF16)
    return hi, lo


def _host_factors(boxes):
    """Mimic the fp32 reference chain, then fold normalization.

    Returns exf [B,N,H] f64, ey [B,N,W] f64, d [B] f64 (offset, ==0 when the
    per-sample min underflows, which it always does for this regime).
    """
    boxes = np.asarray(boxes, np.float32)
    x, y, w, h = boxes[..., 0], boxes[..., 1], boxes[..., 2], boxes[..., 3]
    xc = x + np.float32(np.floor(w / np.float32(2.0)))
    yc = y + np.float32(np.floor(h / np.float32(2.0)))

    gx = np.round(np.linspace(np.float32(0.0), np.float32(H), H, dtype=np.float32))
    gy = np.round(np.linspace(np.float32(0.0), np.float32(W), W, dtype=np.float32))

    # fp32 arithmetic chain exactly like the jax reference
    dx = gx[None, None, :] - xc[..., None]
    dx2 = (dx * dx) / (np.float32(0.25) * w)[..., None]          # f32 [B,N,H]
    dy = gy[None, None, :] - yc[..., None]
    dy2 = (dy * dy) / (np.float32(0.25) * h)[..., None]          # f32 [B,N,W]

    ex = np.exp(np.float64(-0.5) * dx2.astype(np.float64))       # f64 [B,N,H]
    ey = np.exp(np.float64(-0.5) * dy2.astype(np.float64))       # f64 [B,N,W]

    det = (np.float32(0.0625) * w * h).astype(np.float64)        # [B,N]
    factor = (1.0 / (2.0 * math.pi)) * det ** -0.5               # f64 [B,N]

    m_max = factor * ex.max(-1) * ey.max(-1)                     # [B,N]
    m_min = factor * ex.min(-1) * ey.min(-1)
    # cast through f32 so fp32 underflow to 0 is reproduced
    mx = m_max.max(1).astype(np.float32).astype(np.float64)      # [B]
    mn = m_min.min(1).astype(np.float32).astype(np.float64)      # [B]

    a = SCALE_FACTOR / (mx - mn)                                 # [B]
    d = a * mn                                                   # [B]
    exf = a[:, None, None] * factor[..., None] * ex              # f64 [B,N,H]
    return exf, ey, d


def _pack_core_inputs(exf, ey, d, krows=KROWS):
    """Build the per-core packed [KROWS, M*512] bf16 operand arrays.

    Per mask slot m (= sample s * 50 + output channel c, mask n = PERM[c]):
      free [0:128)   lhsT for even rows r=0:  rows k: eh[0::2], el[0::2],
                     eh[0::2], el[0::2], 1, 1
      free [128:256) lhsT for odd rows r=1 (same with [1::2])
      free [256:512) rhs rows k: yh, yh, yl, yl, dh, dl  (dh+dl ~= -d)
    """
    eh, el = _bf16_split(exf)     # [B,N,H] bf16
    yh, yl = _bf16_split(ey)      # [B,N,W]
    dh, dl = _bf16_split(-d)      # [B]

    packed_all = []
    for core in range(N_CORES):
        pk = np.zeros((krows, M_PER_CORE, 512), dtype=BF16)
        for s in range(B_PER_CORE):
            b = core * B_PER_CORE + s
            sl = slice(s * N_MASK, (s + 1) * N_MASK)
            # [N,H] for this sample's masks in output-channel order
            ehb, elb = eh[b][PERM], el[b][PERM]
            yhb, ylb = yh[b][PERM], yl[b][PERM]
            lrows = (ehb, elb, ehb, elb)
            rrows = (yhb, yhb, ylb, ylb)
            for k in range(4):
                pk[k, sl, 0:128] = lrows[k][:, 0::2]
                pk[k, sl, 128:256] = lrows[k][:, 1::2]
                pk[k, sl, 256:512] = rrows[k]
            if krows == 6:
                # offset rows: ones x (-d) split
                pk[4, sl, 0:256] = BF16(1.0)
                pk[5, sl, 0:256] = BF16(1.0)
                pk[4, sl, 256:512] = dh[b]
                pk[5, sl, 256:512] = dl[b]
        packed_all.append(np.ascontiguousarray(pk.reshape(krows, M_PER_CORE * 512)))
    return packed_all


def kernel(boxes, labels, fms_h, fms_w, trace=False, trace_cores=None):
    global LAST_RESULTS
    assert int(np.asarray(fms_h)) == H and int(np.asarray(fms_w)) == W

    labels = np.asarray(labels, np.float32)
    exf, ey, d = _host_factors(boxes)
    # the two offset rows contribute ones * (-d); when every d is 0 (always,
    # for this input regime) they are exact no-ops -- drop them.
    krows = 4 if np.all(d == 0.0) else 6
    packed_all = _pack_core_inputs(exf, ey, d, krows=krows)

    key = ("nc", krows)
    if key not in _NC_CACHE:
        import os

        cfg = {}
        if os.environ.get("KERNEL_CFG"):
            for kv in os.environ["KERNEL_CFG"].split(","):
                k, v = kv.split("=")
                cfg[k] = v.lower() == "true" if v.lower() in ("true", "false") else int(v)
        cfg["krows"] = krows
        _NC_CACHE[key] = (_build_nc(**cfg), cfg.get("stripe", True))
    nc, striped = _NC_CACHE[key]

    if striped:
        # [k, M, 512] -> [2k, M/2, 512]: even masks first k rows, odd next k
        def _stripe(pk):
            pk3 = pk.reshape(krows, M_PER_CORE, 512)
            out = np.concatenate([pk3[:, 0::2], pk3[:, 1::2]], axis=0)
            return np.ascontiguousarray(
                out.reshape(2 * krows, (M_PER_CORE // 2) * 512)
            )

        packed_all = [_stripe(pk) for pk in packed_all]

    in_maps = [{"packed": packed_all[c]} for c in range(N_CORES)]
    kwargs = {}
    if trace:
        kwargs["trace"] = True
        if trace_cores is not None:
            kwargs["trace_cores"] = trace_cores
    try:
        res = run_bass_kernel_spmd(nc, in_maps, core_ids=list(range(N_CORES)), **kwargs)
    except ModuleNotFoundError:
        if not trace:
            raise
        # NTFF profiling hook unavailable in this environment — run untraced.
        res = run_bass_kernel_spmd(nc, in_maps, core_ids=list(range(N_CORES)))
    LAST_RESULTS = res

    attention_masks = np.empty((B, N_MASK, H, W), np.float32)
    for core in range(N_CORES):
        arr = res.results[core]["masks"].reshape(B_PER_CORE, N_MASK, H, W)
        attention_masks[core * B_PER_CORE : (core + 1) * B_PER_CORE] = arr

    attention_labels = labels[:, :, 0][:, PERM].astype(np.float32)
    return attention_masks, attention_labels
